# revision 6
# baseline (speedup 1.0000x reference)
"""CapsuleLayer (dynamic routing) Bass kernel for 8 Trainium2 NeuronCores.

Problem: x (B=32, N=64, d=128), W (1, N, N, D=128, d) fp32.
  u[b,j,k,:] = W[0,j,k] @ x[b,j]          (4.3 GFLOP, W is 268 MB)
  3 rounds of routing: c = softmax_k(b), s = sum_j c*u, v = squash(s),
  b += u*v. Output v (B, N, D, 1).

Sharding: input-capsule axis j split 8 ways (W read exactly once
cluster-wide; softmax over k stays core-local). Per-core partition
layout for u is p = (k%4)*32 + b  (4 column-tiled matmul groups of
M=32), free dims (j=8, k16=k//4, D). The routing capsule-sum over j is
a free-dim reduce locally + AllReduce over cores (iters 0,1) and a
final ReduceScatter (iter 2) so each core finishes squash on its own
shard and writes 1/8 of the output.
"""

import os
import time
from contextlib import ExitStack

import numpy as np

import concourse.mybir as mybir
from concourse import bacc, tile
from concourse.bass_utils import run_bass_kernel_spmd

FP = mybir.dt.float32
AF = mybir.ActivationFunctionType
OP = mybir.AluOpType

B = 32
CORES = 8
ROUTINGS = 3
EPS = 1e-9


def build_nc(N=64, D=128, d=128, verbose=False):
    """Build the SPMD Bass program (identical on all 8 cores)."""
    JPC = N // CORES        # input capsules per core
    KQ = 4                  # k%4 partition groups
    K16 = N // KQ           # k//4 free index
    KCH = 16                # k per W chunk
    NCH = N // KCH          # chunks per j
    assert N % 16 == 0 and D == 128 and d == 128 and B == 32

    nc = bacc.Bacc("TRN2", debug=False, num_devices=CORES, enable_asserts=False)

    w_in = nc.dram_tensor("w", [JPC, N, D, d], FP, kind="ExternalInput").ap()
    x_in = nc.dram_tensor("x", [B, JPC, d], FP, kind="ExternalInput").ap()
    id_in = nc.dram_tensor("ident", [128, 128], FP, kind="ExternalInput").ap()
    a_in = nc.dram_tensor("amat", [128, 128], FP, kind="ExternalInput").ap()
    out_d = nc.dram_tensor("out", [16, K16, D], FP, kind="ExternalOutput").ap()

    with tile.TileContext(nc) as tc, ExitStack() as ctx:
        sb = ctx.enter_context(tc.tile_pool(name="sb", bufs=1))
        wnp = ctx.enter_context(tc.tile_pool(name="wn", bufs=2))
        wtp = ctx.enter_context(tc.tile_pool(name="wt", bufs=2))
        pst = ctx.enter_context(tc.tile_pool(name="pst", bufs=3, space="PSUM"))
        psu = ctx.enter_context(tc.tile_pool(name="psu", bufs=2, space="PSUM"))
        dram = ctx.enter_context(tc.tile_pool(name="dram", bufs=2, space="DRAM"))

        ident = sb.tile([128, 128], FP)
        nc.sync.dma_start(ident[:], id_in[:])
        amat = sb.tile([128, 128], FP)
        nc.sync.dma_start(amat[:], a_in[:])

        # ---- x: (b, j, d) -> xt (d, j, b) via PE transpose
        xn = sb.tile([B, JPC, d], FP)
        nc.sync.dma_start(xn[:], x_in[:])
        xt = sb.tile([128, JPC, B], FP)
        for j in range(JPC):
            psx = pst.tile([128, 512], FP, tag="pst")
            nc.tensor.transpose(psx[:, :B], xn[:, j, :], ident[:B, :B])
            nc.vector.tensor_copy(xt[:, j, :], psx[:, :B])

        # ---- u-phase: stream W, transpose tiles, col-tiled matmuls
        u = sb.tile([128, JPC, K16, D], FP)
        cp_flip = 0
        for j in range(JPC):
            for c in range(NCH):
                wn = wnp.tile([128, KCH, d], FP, tag="wn")
                nc.sync.dma_start(
                    wn[:], w_in[j, c * KCH : (c + 1) * KCH].rearrange("k D d -> D k d")
                )
                # wt layout: (d, kq, g, D) with k = 16c + 4g + kq
                wt = wtp.tile([128, KQ, KCH // KQ, D], FP, tag="wt")
                for g in range(KCH // KQ):
                    ps = pst.tile([128, 512], FP, tag="pst")
                    for t_ in range(KQ):
                        k = KQ * g + t_
                        nc.tensor.transpose(
                            ps[:, 128 * t_ : 128 * (t_ + 1)], wn[:, k, :], ident[:]
                        )
                    if cp_flip % 2 == 0:
                        nc.vector.tensor_copy(wt[:, :, g, :], ps[:])
                    else:
                        nc.scalar.copy(wt[:, :, g, :], ps[:])
                    cp_flip += 1
                psU = psu.tile([128, KQ * 128], FP, tag="psu")
                for kq in range(KQ):
                    nc.tensor.matmul(
                        psU[32 * kq : 32 * (kq + 1), :],
                        xt[:, j, :],
                        wt[:, kq, :, :],
                        start=True,
                        stop=True,
                        tile_position=(0, 32 * kq),
                    )
                if cp_flip % 2 == 0:
                    nc.vector.tensor_copy(u[:, j, KQ * c : KQ * (c + 1), :], psU[:])
                else:
                    nc.scalar.copy(u[:, j, KQ * c : KQ * (c + 1), :], psU[:])
                cp_flip += 1

        # ---- routing
        scr = sb.tile([128, JPC, K16, D], FP)
        Vcum = sb.tile([128, K16, D], FP)
        s_t = sb.tile([128, K16, D], FP)
        Zq = sb.tile([128, JPC, D], FP)
        rZ = sb.tile([128, JPC, D], FP)
        sq = sb.tile([128, K16], FP)
        f0 = sb.tile([128, K16], FP)
        f1 = sb.tile([128, K16], FP)

        def squash_factor(sq_ap, f0_ap, f1_ap, eng_v=None, eng_s=None):
            # f = (sq/(1+sq))/sqrt(sq+eps) computed as sq * 1/((1+sq)*sqrt(sq+eps))
            ev = eng_v or nc.vector
            es = eng_s or nc.scalar
            ev.tensor_scalar_add(f0_ap, sq_ap, EPS)                 # sq+eps
            es.activation(f0_ap, f0_ap, AF.Sqrt)                    # sqrt(sq+eps)
            ev.tensor_scalar_add(f1_ap, sq_ap, 1.0)                 # 1+sq
            ev.tensor_tensor(f1_ap, f1_ap, f0_ap, op=OP.mult)       # (1+sq)*sqrt
            ev.reciprocal(f0_ap, f1_ap)
            ev.tensor_tensor(f0_ap, f0_ap, sq_ap, op=OP.mult)       # f
            return f0_ap

        for it in range(ROUTINGS):
            if it == 0:
                # c uniform: s = (1/N) sum_j u
                nc.vector.tensor_reduce(
                    s_t[:], u[:].transpose([0, 2, 3, 1]), axis=mybir.AxisListType.X,
                    op=OP.add,
                )
                nc.vector.tensor_scalar_mul(s_t[:], s_t[:], 1.0 / N)
            else:
                # t = u * Vcum (broadcast over j); e = exp(t)
                nc.vector.tensor_tensor(
                    scr[:], u[:],
                    Vcum[:, None, :, :].to_broadcast([128, JPC, K16, D]),
                    op=OP.mult,
                )
                nc.scalar.activation(scr[:], scr[:], AF.Exp)
                # Zq = sum_{k16} e  -> (128, JPC, D)
                nc.vector.tensor_reduce(
                    Zq[:], scr[:].transpose([0, 1, 3, 2]), axis=mybir.AxisListType.X,
                    op=OP.add,
                )
                # fold over kq partition groups + replicate: Z = A.T @ Zq
                zq_f = Zq[:].rearrange("p j d -> p (j d)")
                rz_f = rZ[:].rearrange("p j d -> p (j d)")
                zch = min(512, JPC * D)
                for h in range((JPC * D) // zch):
                    psZ = psu.tile([128, zch], FP, tag="psz")
                    nc.tensor.matmul(
                        psZ[:], amat[:], zq_f[:, zch * h : zch * (h + 1)],
                        start=True, stop=True,
                    )
                    nc.vector.reciprocal(rz_f[:, zch * h : zch * (h + 1)], psZ[:])
                # p = (e/Z) * u ; s = sum_j p   (c=e/Z first: e*u can overflow fp32)
                nc.vector.tensor_tensor(
                    scr[:], scr[:],
                    rZ[:, :, None, :].to_broadcast([128, JPC, K16, D]),
                    op=OP.mult,
                )
                nc.vector.tensor_tensor(scr[:], scr[:], u[:], op=OP.mult)
                nc.vector.tensor_reduce(
                    s_t[:], scr[:].transpose([0, 2, 3, 1]), axis=mybir.AxisListType.X,
                    op=OP.add,
                )

            if it < ROUTINGS - 1:
                # AllReduce s over cores, then v = squash(s); Vcum += v
                bi = dram.tile([128, K16, D], FP, tag="cin")
                bo = dram.tile([128, K16, D], FP, tag="cout")
                nc.gpsimd.dma_start(bi[:], s_t[:])
                nc.gpsimd.collective_compute(
                    "AllReduce", OP.add,
                    replica_groups=[list(range(CORES))],
                    ins=[bi[:].opt()], outs=[bo[:].opt()],
                )
                nc.gpsimd.dma_start(s_t[:], bo[:])
                sqr = scr[:, 0, :, :]  # scratch for s^2
                nc.scalar.activation(sqr, s_t[:], AF.Square)
                nc.vector.tensor_reduce(
                    sq[:], sqr, axis=mybir.AxisListType.X, op=OP.add
                )
                f = squash_factor(sq[:], f0[:], f1[:])
                if it == 0:
                    nc.vector.tensor_tensor(
                        Vcum[:], s_t[:],
                        f[:, :, None].to_broadcast([128, K16, D]), op=OP.mult,
                    )
                else:
                    vtmp = scr[:, 1, :, :]
                    nc.vector.tensor_tensor(
                        vtmp, s_t[:],
                        f[:, :, None].to_broadcast([128, K16, D]), op=OP.mult,
                    )
                    nc.vector.tensor_tensor(Vcum[:], Vcum[:], vtmp, op=OP.add)
            else:
                # final: ReduceScatter; each core squashes + writes its shard
                bi = dram.tile([128, K16, D], FP, tag="cin")
                bo = dram.tile([16, K16, D], FP, tag="crs")
                nc.gpsimd.dma_start(bi[:], s_t[:])
                nc.gpsimd.collective_compute(
                    "ReduceScatter", OP.add,
                    replica_groups=[list(range(CORES))],
                    ins=[bi[:].opt()], outs=[bo[:].opt()],
                )
                ssh = sb.tile([16, K16, D], FP)
                nc.gpsimd.dma_start(ssh[:], bo[:])
                sq16 = sb.tile([16, K16], FP)
                f016 = sb.tile([16, K16], FP)
                f116 = sb.tile([16, K16], FP)
                s2 = sb.tile([16, K16, D], FP)
                nc.scalar.activation(s2[:], ssh[:], AF.Square)
                nc.vector.tensor_reduce(
                    sq16[:], s2[:], axis=mybir.AxisListType.X, op=OP.add
                )
                f = squash_factor(sq16[:], f016[:], f116[:])
                nc.vector.tensor_tensor(
                    s2[:], ssh[:],
                    f[:, :, None].to_broadcast([16, K16, D]), op=OP.mult,
                )
                nc.sync.dma_start(out_d[:], s2[:])

    nc.compile()
    return nc


def make_const_inputs():
    ident = np.eye(128, dtype=np.float32)
    p = np.arange(128)
    amat = (p[:, None] % 32 == p[None, :] % 32).astype(np.float32)
    return ident, amat


def make_in_maps(x, W, N=64):
    """Split full inputs into per-core in_maps."""
    JPC = N // CORES
    ident, amat = make_const_inputs()
    in_maps = []
    for c in range(CORES):
        in_maps.append(
            {
                "w": np.ascontiguousarray(W[0, c * JPC : (c + 1) * JPC]),
                "x": np.ascontiguousarray(x[:, c * JPC : (c + 1) * JPC, :]),
                "ident": ident,
                "amat": amat,
            }
        )
    return in_maps


def assemble_output(results, N=64, D=128):
    """Gather per-core RS shards back into (B, N, D, 1)."""
    K16 = N // 4
    v = np.zeros((B, N, D), dtype=np.float32)
    for c in range(CORES):
        sh = results[c]["out"]  # (16, K16, D); global rows 16c..16c+16
        kq = c // 2
        b0 = 16 * (c % 2)
        v[b0 : b0 + 16, kq::4, :] = sh
    return v[:, :, :, None]


_NC_CACHE = {}


def kernel(x: np.ndarray, W: np.ndarray) -> np.ndarray:
    N, D, d = W.shape[1], W.shape[3], W.shape[4]
    key = (N, D, d)
    if key not in _NC_CACHE:
        _NC_CACHE[key] = build_nc(N=N, D=D, d=d)
    nc = _NC_CACHE[key]
    in_maps = make_in_maps(x, W, N=N)
    r = run_bass_kernel_spmd(nc, in_maps, list(range(CORES)))
    return assemble_output(r.results, N=N, D=D)


# revision 27
# speedup vs baseline: 1.3843x; 1.3843x over previous
"""CapsuleLayer (dynamic routing) Bass kernel for 8 Trainium2 NeuronCores.

Problem: x (B=32, N=64, d=128), W (1, N, N, D=128, d) fp32.
  u[b,j,k,:] = W[0,j,k] @ x[b,j]          (4.3 GFLOP, W is 268 MB)
  3 rounds of routing: c = softmax_k(b), s = sum_j c*u, v = squash(s),
  b += u*v. Output v (B, N, D, 1).

Sharding: input-capsule axis j split 8 ways (W read exactly once
cluster-wide; softmax over k stays core-local). Per-core partition
layout for u is p = (k%4)*32 + b  (4 column-tiled matmul groups of
M=32), free dims (j=8, k16=k//4, D). The routing capsule-sum over j is
a free-dim reduce locally + AllReduce over cores (iters 0,1) and a
final ReduceScatter (iter 2) so each core finishes squash on its own
shard and writes 1/8 of the output.
"""

import os
import time
from contextlib import ExitStack

import numpy as np

import concourse.mybir as mybir
from concourse import bacc, tile
from concourse.bass_utils import run_bass_kernel_spmd

FP = mybir.dt.float32
FR = mybir.dt.float32r
AF = mybir.ActivationFunctionType
OP = mybir.AluOpType

B = 32
CORES = 8
ROUTINGS = 3
EPS = 1e-9


def build_nc(N=64, D=128, d=128, verbose=False, collectives=True, f32r=True):
    """Build the SPMD Bass program (identical on all 8 cores).

    collectives=False replaces the cross-core collectives with local DMA
    copies (identical dataflow, wrong math) so the single-core
    TimelineSim can estimate kernel time; add documented collective
    latencies (~2x24us AR + 17us RS) on top.
    """
    JPC = N // CORES        # input capsules per core
    KQ = 4                  # k%4 partition groups
    K16 = N // KQ           # k//4 free index
    KCH = min(32, N)        # k per W DMA chunk
    NCH = N // KCH          # chunks per j
    NMG = KCH // 16         # 16-k matmul groups per chunk
    assert N % 16 == 0 and D == 128 and d == 128 and B == 32

    nc = bacc.Bacc("TRN2", debug=False, num_devices=CORES, enable_asserts=False)

    w_in = nc.dram_tensor("w", [JPC, N, D, d], FP, kind="ExternalInput").ap()
    x_in = nc.dram_tensor("x", [B, JPC, d], FP, kind="ExternalInput").ap()
    id_in = nc.dram_tensor("ident", [128, 128], FP, kind="ExternalInput").ap()
    a_in = nc.dram_tensor("amat", [128, 128], FP, kind="ExternalInput").ap()
    out_d = nc.dram_tensor("out", [16, K16, D], FP, kind="ExternalOutput").ap()

    with tile.TileContext(nc) as tc, ExitStack() as ctx:
        sb = ctx.enter_context(tc.tile_pool(name="sb", bufs=1))
        wnp = ctx.enter_context(tc.tile_pool(name="wn", bufs=2))
        wtp = ctx.enter_context(tc.tile_pool(name="wt", bufs=2))
        pst = ctx.enter_context(tc.tile_pool(name="pst", bufs=3, space="PSUM"))
        psu = ctx.enter_context(tc.tile_pool(name="psu", bufs=2, space="PSUM"))
        dram = ctx.enter_context(tc.tile_pool(name="dram", bufs=2, space="DRAM"))

        ident = sb.tile([128, 128], FP)
        nc.sync.dma_start(ident[:], id_in[:])
        amat = sb.tile([128, 128], FP)
        nc.sync.dma_start(amat[:], a_in[:])

        # ---- x: (b, j, d) -> xt (d, j, b) via PE transpose
        xn = sb.tile([B, JPC, d], FP)
        nc.sync.dma_start(xn[:], x_in[:])
        # r(): view an AP as f32r for the PE; rc(): cast-on-copy producer
        def r(ap):
            return ap.bitcast(FR) if f32r else ap

        # xt_pad[j][p, kq, m] = x_t[j][p, m-32kq] for m in [32kq, 32kq+32), else 0.
        # Each kq's matmul then has M=128 with only its quadrant nonzero, so 4
        # accumulating matmuls assemble u on partitions (k%4)*32+b without
        # tile_position (f32r + tile_position is rejected by codegen).
        xtp = ctx.enter_context(tc.tile_pool(name="xtp", bufs=2))

        def make_xt_pad(j):
            xt_pad = xtp.tile([128, KQ, 128], FP, tag="xt_pad")
            nc.vector.memset(xt_pad[:], 0.0)
            psx = pst.tile([128, 512], FP, tag="pst")
            nc.tensor.transpose(psx[:, :B], xn[:, j, :], ident[:B, :B])
            for kq in range(KQ):
                nc.vector.tensor_copy(
                    r(xt_pad[:, kq, 32 * kq : 32 * (kq + 1)]), psx[:, :B]
                )
            return xt_pad

        # ---- u-phase: stream W, transpose tiles, col-tiled matmuls
        u = sb.tile([128, JPC, K16, D], FP)
        cp_flip = 0
        for j in range(JPC):
            xt_pad = make_xt_pad(j)
            for c in range(NCH):
                wn = wnp.tile([128, KCH, d], FP, tag="wn")
                nc.sync.dma_start(
                    wn[:], w_in[j, c * KCH : (c + 1) * KCH].rearrange("k D d -> D k d")
                )
                for h in range(NMG):
                    # wt layout: (d, kq, g, D) with k = KCH*c + 16h + 4g + kq
                    wt = wtp.tile([128, KQ, 4, D], FP, tag="wt")
                    for g in range(4):
                        ps = pst.tile([128, 512], FP, tag="pst")
                        for t_ in range(KQ):
                            k = 16 * h + KQ * g + t_
                            nc.tensor.transpose(
                                ps[:, 128 * t_ : 128 * (t_ + 1)], wn[:, k, :],
                                ident[:],
                            )
                        if cp_flip % 2 == 0:
                            nc.vector.tensor_copy(r(wt[:, :, g, :]), ps[:])
                        else:
                            nc.scalar.copy(r(wt[:, :, g, :]), ps[:])
                        cp_flip += 1
                    psU = psu.tile([128, KQ * 128], FP, tag="psu")
                    for kq in range(KQ):
                        nc.tensor.matmul(
                            psU[:],
                            r(xt_pad[:, kq, :]),
                            r(wt[:, kq, :, :]),
                            start=(kq == 0),
                            stop=(kq == KQ - 1),
                        )
                    k16_0 = (KCH * c + 16 * h) // KQ
                    if cp_flip % 2 == 0:
                        nc.vector.tensor_copy(u[:, j, k16_0 : k16_0 + 4, :], psU[:])
                    else:
                        nc.scalar.copy(u[:, j, k16_0 : k16_0 + 4, :], psU[:])
                    cp_flip += 1

        # ---- routing
        scr = sb.tile([128, JPC, K16, D], FP)
        Vcum = sb.tile([128, K16, D], FP)
        s_t = sb.tile([128, K16, D], FP)
        Zq = sb.tile([128, JPC, D], FP)
        rZ = Zq  # reciprocal overwrites the fold input (dead after the matmul)
        sq = sb.tile([128, K16], FP)
        f0 = sb.tile([128, K16], FP)
        f1 = sb.tile([128, K16], FP)

        def squash_factor(sq_ap, f0_ap, f1_ap, eng_v=None, eng_s=None):
            # f = (sq/(1+sq))/sqrt(sq+eps) computed as sq * 1/((1+sq)*sqrt(sq+eps))
            ev = eng_v or nc.vector
            es = eng_s or nc.scalar
            ev.tensor_scalar_add(f0_ap, sq_ap, EPS)                 # sq+eps
            es.activation(f0_ap, f0_ap, AF.Sqrt)                    # sqrt(sq+eps)
            ev.tensor_scalar_add(f1_ap, sq_ap, 1.0)                 # 1+sq
            ev.tensor_tensor(f1_ap, f1_ap, f0_ap, op=OP.mult)       # (1+sq)*sqrt
            ev.reciprocal(f0_ap, f1_ap)
            ev.tensor_tensor(f0_ap, f0_ap, sq_ap, op=OP.mult)       # f
            return f0_ap

        # engine split of the j axis for the big elementwise passes:
        # DVE gets j [0, JSP), GpSimd (2x slower) gets [JSP, JPC)
        JSP = max(1, (JPC * 5) // 8) if JPC > 1 else 1
        jsplits = [(nc.vector, 0, JSP)]
        if JSP < JPC:
            jsplits.append((nc.gpsimd, JSP, JPC))

        def big_tt(out_t, in0_t, in1_ap_fn):
            for eng, j0, j1 in jsplits:
                eng.tensor_tensor(
                    out_t[:, j0:j1], in0_t[:, j0:j1], in1_ap_fn(j0, j1), op=OP.mult
                )

        for it in range(ROUTINGS):
            if it == 0:
                # c uniform: s = (1/N) sum_j u
                for h in range(2):
                    sl = slice(h * K16 // 2, (h + 1) * K16 // 2)
                    nc.vector.tensor_reduce(
                        s_t[:, sl], u[:, :, sl].transpose([0, 2, 3, 1]),
                        axis=mybir.AxisListType.X, op=OP.add,
                    )
                nc.vector.tensor_scalar_mul(s_t[:], s_t[:], 1.0 / N)
            else:
                # t = u * Vcum (broadcast over j); e = exp(t)
                big_tt(
                    scr, u,
                    lambda j0, j1: Vcum[:, None, :, :].to_broadcast(
                        [128, j1 - j0, K16, D]
                    ),
                )
                for eng, j0, j1 in jsplits:
                    nc.scalar.activation(scr[:, j0:j1], scr[:, j0:j1], AF.Exp)
                # Zq = sum_{k16} e  -> (128, JPC, D)
                for eng, j0, j1 in jsplits:
                    nc.vector.tensor_reduce(
                        Zq[:, j0:j1], scr[:, j0:j1].transpose([0, 1, 3, 2]),
                        axis=mybir.AxisListType.X, op=OP.add,
                    )
                # fold over kq partition groups + replicate: Z = A.T @ Zq
                zq_f = Zq[:].rearrange("p j d -> p (j d)")
                rz_f = rZ[:].rearrange("p j d -> p (j d)")
                zch = min(512, JPC * D)
                for h in range((JPC * D) // zch):
                    psZ = psu.tile([128, zch], FP, tag="psz")
                    nc.tensor.matmul(
                        psZ[:], amat[:], zq_f[:, zch * h : zch * (h + 1)],
                        start=True, stop=True,
                    )
                    nc.vector.reciprocal(rz_f[:, zch * h : zch * (h + 1)], psZ[:])
                # p = (e/Z) * u ; s = sum_j p   (c=e/Z first: e*u can overflow fp32)
                big_tt(
                    scr, scr,
                    lambda j0, j1: rZ[:, j0:j1, None, :].to_broadcast(
                        [128, j1 - j0, K16, D]
                    ),
                )
                big_tt(scr, scr, lambda j0, j1: u[:, j0:j1])
                for h in range(2):
                    sl = slice(h * K16 // 2, (h + 1) * K16 // 2)
                    nc.vector.tensor_reduce(
                        s_t[:, sl], scr[:, :, sl].transpose([0, 2, 3, 1]),
                        axis=mybir.AxisListType.X, op=OP.add,
                    )

            if it < ROUTINGS - 1:
                # AllReduce s over cores, then v = squash(s); Vcum += v
                bi = dram.tile([128, K16, D], FP, tag="cin")
                bo = dram.tile([128, K16, D], FP, tag="cout")
                nc.sync.dma_start(bi[:], s_t[:])
                if collectives:
                    nc.gpsimd.collective_compute(
                        "AllReduce", OP.add,
                        replica_groups=[list(range(CORES))],
                        ins=[bi[:].opt()], outs=[bo[:].opt()],
                    )
                else:
                    nc.gpsimd.dma_start(bo[:], bi[:])
                nc.sync.dma_start(s_t[:], bo[:])
                sqr = scr[:, 0, :, :]  # scratch for s^2
                nc.scalar.activation(sqr, s_t[:], AF.Square)
                nc.vector.tensor_reduce(
                    sq[:], sqr, axis=mybir.AxisListType.X, op=OP.add
                )
                f = squash_factor(sq[:], f0[:], f1[:])
                if it == 0:
                    nc.vector.tensor_tensor(
                        Vcum[:], s_t[:],
                        f[:, :, None].to_broadcast([128, K16, D]), op=OP.mult,
                    )
                else:
                    vtmp = scr[:, 1, :, :]
                    nc.vector.tensor_tensor(
                        vtmp, s_t[:],
                        f[:, :, None].to_broadcast([128, K16, D]), op=OP.mult,
                    )
                    nc.vector.tensor_tensor(Vcum[:], Vcum[:], vtmp, op=OP.add)
            else:
                # final: ReduceScatter; each core squashes + writes its shard
                bi = dram.tile([128, K16, D], FP, tag="cin")
                bo = dram.tile([16, K16, D], FP, tag="crs")
                nc.sync.dma_start(bi[:], s_t[:])
                if collectives:
                    nc.gpsimd.collective_compute(
                        "ReduceScatter", OP.add,
                        replica_groups=[list(range(CORES))],
                        ins=[bi[:].opt()], outs=[bo[:].opt()],
                    )
                else:
                    nc.gpsimd.dma_start(bo[:], bi[:16])
                ssh = s_t[:16]
                nc.sync.dma_start(ssh, bo[:])
                s2 = scr[:16, 0]
                nc.scalar.activation(s2, ssh, AF.Square)
                nc.vector.tensor_reduce(
                    sq[:16], s2, axis=mybir.AxisListType.X, op=OP.add
                )
                f = squash_factor(sq[:16], f0[:16], f1[:16])
                nc.vector.tensor_tensor(
                    s2, ssh,
                    f[:, :, None].to_broadcast([16, K16, D]), op=OP.mult,
                )
                nc.sync.dma_start(out_d[:], s2)

    nc.compile()
    return nc


def make_const_inputs():
    ident = np.eye(128, dtype=np.float32)
    p = np.arange(128)
    amat = (p[:, None] % 32 == p[None, :] % 32).astype(np.float32)
    return ident, amat


def make_in_maps(x, W, N=64):
    """Split full inputs into per-core in_maps."""
    JPC = N // CORES
    ident, amat = make_const_inputs()
    in_maps = []
    for c in range(CORES):
        in_maps.append(
            {
                "w": np.ascontiguousarray(W[0, c * JPC : (c + 1) * JPC]),
                "x": np.ascontiguousarray(x[:, c * JPC : (c + 1) * JPC, :]),
                "ident": ident,
                "amat": amat,
            }
        )
    return in_maps


def assemble_output(results, N=64, D=128):
    """Gather per-core RS shards back into (B, N, D, 1)."""
    K16 = N // 4
    v = np.zeros((B, N, D), dtype=np.float32)
    for c in range(CORES):
        sh = results[c]["out"]  # (16, K16, D); global rows 16c..16c+16
        kq = c // 2
        b0 = 16 * (c % 2)
        v[b0 : b0 + 16, kq::4, :] = sh
    return v[:, :, :, None]


_NC_CACHE = {}


def kernel(x: np.ndarray, W: np.ndarray) -> np.ndarray:
    N, D, d = W.shape[1], W.shape[3], W.shape[4]
    key = (N, D, d)
    if key not in _NC_CACHE:
        _NC_CACHE[key] = build_nc(N=N, D=D, d=d)
    nc = _NC_CACHE[key]
    in_maps = make_in_maps(x, W, N=N)
    r = run_bass_kernel_spmd(nc, in_maps, list(range(CORES)))
    return assemble_output(r.results, N=N, D=D)
